# revision 40
# baseline (speedup 1.0000x reference)
"""MHSA Trainium2 kernel v2: B=4, S=2048, D=1024, H=16 heads of 64.

Sharding (8 cores): core c -> batch b=c//2, head-group g=c%2 (8 heads = 512
proj cols). Identical program on every core; only the data differs.

v2 vs baseline (f32r, 588us):
  - bf16 matmul operands everywhere -> FWL (fast weight load) engages,
    killing the 189ns/mm LDWEIGHTS tax; DMA bytes halve.
  - scores (K=64) matmuls PE-row-tiled: head 2jt on array rows 0-63,
    head 2jt+1 on rows 64-127 run CONCURRENTLY (tile_position auto-derived
    from base partitions) -> ~2x on the score GEMMs.
  - exp batched into N=2048/N=1024 PSUM-source ACTIVATEs (asym [2kt|1kt]
    supercycles) to amortize the ~350cy ACT fixed cost; PSUM budget is
    exactly 8 banks: s4(4) + s2(2, shared with kq/v/oproj psum) + pvA + pvB.
  - PV lhsT padded to 128 cols (V|ones|zeros) so PV also gets FWL.
  - software pipelining: PV mms for supercycle g emitted in g+1 so the PE
    never head-of-line blocks on the ACT; kq(jt1..3), v-pass and out-proj
    run as PE filler inside the ACT-bound attention blocks.

Per-core layouts (host pre-transposes; no on-device transposes):
  xT  [1024, 2048] = x[b].T          wqT/wkT/wvT [1024, 512] = W[cols].T
  woT [512, 1024]  = Wo[:, cols].T   out [2048, 1024] partial (host sums pairs)
  (1/8 score scale pre-folded into wqT/bq on host; v bias deferred:
   bv @ woT added on host.)
"""

import os
from collections import defaultdict, deque
from contextlib import ExitStack

import numpy as np

import concourse.bass as bass
import concourse.mybir as mybir


def _install_ntff_shim():
    """The agent image's `antenv` lacks `axon_hooks`, which
    run_bass_kernel_spmd imports when trace=True under axon. Provide it,
    wired to the ctypes NTFF hook from trn_agent_boot when available."""
    import sys
    import types
    try:
        from antenv import axon_hooks  # noqa: F401
        return
    except ImportError:
        pass
    try:
        mod = types.ModuleType("antenv.axon_hooks")
        mod._hook = None
        mod.set_axon_ntff_profile_hook = lambda h: setattr(mod, "_hook", h)
        mod.get_axon_ntff_profile_hook = lambda: mod._hook
        import antenv
        sys.modules["antenv.axon_hooks"] = mod
        antenv.axon_hooks = mod
        try:
            from trn_agent_boot.trn_boot import _ntff_profile_via_ctypes
            import os.path
            so = "/opt/axon/libaxon_pjrt.so"
            if os.path.exists(so):
                mod._hook = _ntff_profile_via_ctypes(so)
        except Exception:
            pass
    except Exception:
        pass


_install_ntff_shim()
import concourse.tile as tile
from concourse import bacc
from concourse.bass_utils import run_bass_kernel_spmd

F32 = mybir.dt.float32
BF16 = mybir.dt.bfloat16

S = 2048       # sequence (rows per core's batch)
DF = 1024      # full model dim (contraction for projections)
J = 512        # proj cols per core (8 heads x 64)
HEADS = 8
HD = 64
N_CORES = 8
EXP = mybir.ActivationFunctionType.Exp

LAST_RESULT = {}

# supercycle kt-grouping: [2kt -> s4/ACT-2048 | 1kt -> s2/ACT-1024]
GROUPS = [(0, 1, 2), (3, 4, 5), (6, 7, 8), (9, 10, 11), (12, 13, 14), (15,)]


def _build():
    nc = bacc.Bacc(None, target_bir_lowering=False, debug=False)

    # packed layouts: [128 partitions, kc, ...] so each loads in ~1 DMA
    xT_d = nc.declare_dram_parameter("xTp", [128, 8, S], BF16, False)
    wqT_d = nc.declare_dram_parameter("wqTp", [128, 8 * J], BF16, False)
    wkT_d = nc.declare_dram_parameter("wkTp", [128, 8 * J], BF16, False)
    wvT_d = nc.declare_dram_parameter("wvTp", [128, 8 * J], BF16, False)
    bq_d = nc.declare_dram_parameter("bq", [J], F32, False)
    bk_d = nc.declare_dram_parameter("bk", [J], F32, False)
    woT_d = nc.declare_dram_parameter("woTp", [128, 4 * DF], BF16, False)
    out_d = nc.declare_dram_parameter("out", [S, DF], BF16, isOutput=True)

    with tile.TileContext(nc) as tc, ExitStack() as ctx:
        persist = ctx.enter_context(tc.tile_pool(name="persist", bufs=1))
        ps = ctx.enter_context(tc.tile_pool(name="ps", bufs=1, space="PSUM"))
        ptp = ctx.enter_context(tc.tile_pool(name="ptp", bufs=2))
        tmp = ctx.enter_context(tc.tile_pool(name="tmp", bufs=2))
        recp = ctx.enter_context(tc.tile_pool(name="recp", bufs=1))
        dscr = ctx.enter_context(tc.tile_pool(name="dscr", bufs=1, space="DRAM"))

        xt_all = persist.tile([128, 8, S], BF16, name="xt", tag="xt")
        wq_all = persist.tile([128, 8 * J], BF16, name="wqa", tag="wqa")
        wk_all = persist.tile([128, 8 * J], BF16, name="wka", tag="wka")
        wv_all = persist.tile([128, 8 * J], BF16, name="wva", tag="wva")
        wo_all = persist.tile([128, 4 * DF], BF16, name="woa", tag="woa")
        qT = [persist.tile([128, S], BF16, name=f"qT{i}", tag=f"qT{i}")
              for i in range(4)]
        kT = [persist.tile([128, S], BF16, name=f"kT{i}", tag=f"kT{i}")
              for i in range(4)]
        # V padded to 128 cols per head: [v(64) | ones(1) | zeros(63)] -> FWL
        vt = [persist.tile([128, HEADS, 128], BF16, name=f"v{i}", tag=f"v{i}")
              for i in range(16)]
        attnT = [persist.tile([128, S], BF16, name=f"at{i}", tag=f"at{i}")
                 for i in range(4)]
        bq_sb = persist.tile([128, 4], F32, name="bq", tag="bq")
        bk_sb = persist.tile([128, 4], F32, name="bk", tag="bk")

        rd = dscr.tile([HEADS, S], F32, name="rd", tag="rd")
        rd2 = dscr.tile([HEADS, S], F32, name="rd2", tag="rd2")

        # DMA order favors time-to-first-matmul: wk + bias + x(sc0) first.
        # Packed layouts -> one big DMA per tensor (queue-overhead bound
        # otherwise). All on the sync queue: scalar-queue DMAs would sit in
        # front of every ACTIVATE in that in-order queue.
        nc.sync.dma_start(out=wk_all[:, 0:2048], in_=wkT_d[:, 0:2048])
        nc.sync.dma_start(out=xt_all[:, 0:4, 0:512], in_=xT_d[:, 0:4, 0:512])
        nc.sync.dma_start(out=bq_sb, in_=bq_d[:].rearrange("(a p) -> p a", p=128))
        nc.sync.dma_start(out=bk_sb, in_=bk_d[:].rearrange("(a p) -> p a", p=128))
        nc.sync.dma_start(out=wk_all[:, 2048:4096], in_=wkT_d[:, 2048:4096])
        nc.sync.dma_start(out=xt_all[:, 4:8, 0:512], in_=xT_d[:, 4:8, 0:512])
        nc.sync.dma_start(out=wq_all, in_=wqT_d[:, :])
        for sc in range(1, 4):
            ss = slice(512 * sc, 512 * (sc + 1))
            nc.sync.dma_start(out=xt_all[:, :, ss], in_=xT_d[:, :, ss])
            if sc == 1:
                nc.sync.dma_start(out=wv_all, in_=wvT_d[:, :])
        nc.sync.dma_start(out=wo_all, in_=woT_d[:, :])
        for kt in range(16):
            nc.gpsimd.memset(vt[kt][:, :, HD:HD + 1], 1.0)
            nc.gpsimd.memset(vt[kt][:, :, HD + 1:128], 0.0)

        def kq_group(jt, sc, which):
            p = ps.tile([128, 512], F32, name="kqps", tag="s2",
                        padded_shape=[128, 1024])
            w, dst, b = ((wk_all, kT, bk_sb) if which == "k"
                         else (wq_all, qT, bq_sb))
            ss = slice(512 * sc, 512 * (sc + 1))
            for kc in range(8):
                jj = slice(512 * kc + 128 * jt, 512 * kc + 128 * (jt + 1))
                nc.tensor.matmul(p, w[:, jj], xt_all[:, kc, ss],
                                 start=(kc == 0), stop=(kc == 7))
            nc.vector.tensor_scalar_add(dst[jt][:, ss], p, b[:, jt:jt + 1])

        def v_group(kt):
            p = ps.tile([128, 512], F32, name="vps", tag="s2",
                        padded_shape=[128, 1024])
            tt = slice(128 * kt, 128 * (kt + 1))
            for kc in range(8):
                nc.tensor.matmul(p, xt_all[:, kc, tt],
                                 wv_all[:, 512 * kc:512 * (kc + 1)],
                                 start=(kc == 0), stop=(kc == 7))
            nc.vector.tensor_copy(
                vt[kt][:, :, 0:HD], p[:].rearrange("p (h d) -> p h d", h=HEADS))

        OPADS = {"s4": [128, 2048], "s2": [128, 1024],
                 "pvA": [128, 512], "pvB": [128, 512]}

        def oproj_piece(st, oc, tag):
            p = ps.tile([128, 512], F32, name="ops", tag=tag,
                        padded_shape=OPADS[tag])
            sl = slice(128 * st, 128 * (st + 1))
            ocs = slice(512 * oc, 512 * (oc + 1))
            for jc in range(4):
                nc.tensor.matmul(
                    p, attnT[jc][:, sl],
                    wo_all[:, 1024 * jc + 512 * oc:1024 * jc + 512 * (oc + 1)],
                    start=(jc == 0), stop=(jc == 3))
            osb = tmp.tile([128, 512], BF16, name="osb", tag="osb")
            nc.vector.tensor_copy(osb, p)
            nc.sync.dma_start(out=out_d[sl, ocs], in_=osb)

        def oproj_piece2(st, tag):
            """Both oc halves of one token slice; consecutive mms share
            lhsT (attnT[jc]) so the weight load can be elided."""
            p = ps.tile([128, 1024], F32, name="ops2", tag=tag,
                        padded_shape=OPADS[tag])
            sl = slice(128 * st, 128 * (st + 1))
            for jc in range(4):
                for oc in range(2):
                    nc.tensor.matmul(
                        p[:, 512 * oc:512 * (oc + 1)], attnT[jc][:, sl],
                        wo_all[:, 1024 * jc + 512 * oc:1024 * jc + 512 * (oc + 1)],
                        start=(jc == 0), stop=(jc == 3))
            osb2 = tmp.tile([128, 1024], BF16, name="osb2", tag="osb2")
            nc.vector.tensor_copy(osb2, p)
            nc.sync.dma_start(out=out_d[sl, :], in_=osb2)

        def norm(sq):
            s0 = 512 * sq
            # the last column's norm chain rides the scalar DMA queue: the
            # final ACTIVATE precedes it there, and it skips the tail's
            # out-store traffic on the sync queue
            dq = nc.scalar if sq == 3 else nc.sync
            srows = tmp.tile([HEADS, 512], F32, name="srows", tag="srows")
            dq.dma_start(out=srows, in_=rd[:, s0:s0 + 512])
            rrec = tmp.tile([HEADS, 512], F32, name="rrec", tag="rrec")
            nc.vector.reciprocal_approx_fast(rrec, srows)
            dq.dma_start(out=rd2[:, s0:s0 + 512], in_=rrec)
            for jt in range(4):
                rec = recp.tile([128, 512], F32, name=f"rec{jt}", tag=f"rec{jt}")
                dq.dma_start(
                    out=rec[0:64, :],
                    in_=rd2[2 * jt:2 * jt + 1, s0:s0 + 512].partition_broadcast(64))
                dq.dma_start(
                    out=rec[64:128, :],
                    in_=rd2[2 * jt + 1:2 * jt + 2, s0:s0 + 512].partition_broadcast(64))
                nc.gpsimd.tensor_mul(attnT[jt][:, s0:s0 + 512],
                                     attnT[jt][:, s0:s0 + 512], rec)

        def score_pair(dst_ps, col, jt, kt, s0):
            """Row-tiled concurrent pair: head 2jt on rows 0-63, head 2jt+1
            on rows 64-127 (tile_position auto from base partitions)."""
            tt = slice(128 * kt, 128 * (kt + 1))
            nc.tensor.matmul(dst_ps[:, col:col + 512],
                             kT[jt][0:64, tt], qT[jt][0:64, s0:s0 + 512])
            nc.tensor.matmul(dst_ps[:, col + 512:col + 1024],
                             kT[jt][64:128, tt], qT[jt][64:128, s0:s0 + 512])

        def block(jt, sq, fillers, pops_per_sc, oproj_q=None):
            s0 = 512 * sq
            pv = [None, None]   # lazy: allocated at first emit_pv, so that
            # supercycle-0 oproj pops can reuse the pv banks WAR-cleanly
            pend = []

            def emit_pv(kts_p, pt):
                if pv[0] is None:
                    pv[0] = ps.tile([128, 512], F32, name="pvA", tag="pvA")
                    pv[1] = ps.tile([128, 512], F32, name="pvB", tag="pvB")
                for i, kt in enumerate(kts_p):
                    nc.tensor.matmul(pv[0], vt[kt][:, 2 * jt, :],
                                     pt[:, 1024 * i:1024 * i + 512],
                                     start=(kt == 0), stop=(kt == 15))
                    nc.tensor.matmul(pv[1], vt[kt][:, 2 * jt + 1, :],
                                     pt[:, 1024 * i + 512:1024 * (i + 1)],
                                     start=(kt == 0), stop=(kt == 15))

            for gi, kts in enumerate(GROUPS):
                new = []
                if len(kts) >= 2:
                    s4 = ps.tile([128, 2048], F32, name="s4", tag="s4")
                    for i, kt in enumerate(kts[:2]):
                        score_pair(s4, 1024 * i, jt, kt, s0)
                    pt4 = ptp.tile([128, 2048], BF16, name="pt4", tag="pt4")
                    nc.scalar.activation(pt4, s4, EXP)
                    new.append((kts[:2], pt4))
                if len(kts) in (1, 3):
                    kt2 = kts[-1]
                    s2 = ps.tile([128, 1024], F32, name="s2", tag="s2")
                    score_pair(s2, 0, jt, kt2, s0)
                    pt2 = ptp.tile([128, 1024], BF16, name="pt2", tag="pt2")
                    nc.scalar.activation(pt2, s2, EXP)
                    new.append(((kt2,), pt2))
                if gi == 0 and oproj_q:
                    for tg in ("pvA", "pvB", "pvA", "pvB"):
                        if oproj_q:
                            st, oc = oproj_q.popleft()
                            oproj_piece(st, oc, tg)
                for kts_p, pt in pend:
                    emit_pv(kts_p, pt)
                pend = new
                for _ in range(pops_per_sc):
                    if fillers:
                        fillers.popleft()()
            for kts_p, pt in pend:
                emit_pv(kts_p, pt)
            pvA, pvB = pv

            nc.vector.tensor_copy(attnT[jt][0:64, s0:s0 + 512], pvA[0:64, :])
            nc.vector.tensor_copy(attnT[jt][64:128, s0:s0 + 512], pvB[0:64, :])
            for h, pv in ((0, pvA), (1, pvB)):
                sr = tmp.tile([1, 512], F32, name="sr", tag="sr")
                nc.vector.tensor_copy(sr, pv[64:65, :])
                nc.sync.dma_start(out=rd[2 * jt + h:2 * jt + h + 1, s0:s0 + 512],
                                  in_=sr)

        # ---- emission (pair-major for even PE-filler spread; p2/p3
        # interleaved by sq so each query column's out-projection becomes
        # ready early enough to hide at supercycle-0 pops) ----
        # minimal pre-block work so the ACT starts ASAP: k(all sc) + q(sc0)
        for sc in range(4):
            kq_group(0, sc, "k")
        kq_group(0, 0, "q")

        def kqf(jt, sc, w):
            return lambda: kq_group(jt, sc, w)

        # v-pass woven into block(0,0): v(kt) groups must precede PV(kt).
        # 3 v-groups per supercycle pop keep v(kt) ahead of the
        # one-supercycle-deferred PV. q(sc1) rides last for block(0,1).
        vq = deque(range(16))

        def v_filler():
            for _ in range(3):
                if vq:
                    v_group(vq.popleft())

        b00 = deque([v_filler] * 6)
        b00.append(kqf(0, 1, "q"))
        block(0, 0, b00, 2)
        while vq:
            v_group(vq.popleft())

        fillers = deque([kqf(0, 2, "q"), kqf(0, 3, "q")]
                        + [kqf(jt, sc, w) for jt in (1, 2, 3)
                           for sc in range(4) for w in ("k", "q")])
        for sq in (1, 2, 3):
            block(0, sq, fillers, 1)
        for sq in range(4):
            block(1, sq, fillers, 2)
        while fillers:
            fillers.popleft()()

        oq = deque()
        empty = deque()
        for sq in range(4):
            block(2, sq, empty, 1)
            # pops only on the (3,*) block: one full block of settle time
            # for the previous column's norm chain
            block(3, sq, empty, 1, oproj_q=oq)
            norm(sq)
            for st_l in range(4):
                for oc in range(2):
                    oq.append((4 * sq + st_l, oc))
        # tail: pair both oc halves per st into [128,1024] pieces, 2-deep
        # rotation over the freed wide psum tags
        rem = defaultdict(list)
        while oq:
            st, oc = oq.popleft()
            rem[st].append(oc)
        rots2 = ("s4", "s2")
        r = 0
        for st in sorted(rem):
            if len(rem[st]) == 2:
                oproj_piece2(st, rots2[r % 2])
                r += 1
            else:
                for oc in rem[st]:
                    oproj_piece(st, oc, rots2[r % 2])
                    r += 1
    nc.compile()
    return nc


_NC_CACHE = {}


def _get_nc():
    if "nc" not in _NC_CACHE:
        _NC_CACHE["nc"] = _build()
    return _NC_CACHE["nc"]


def kernel(**inputs):
    import ml_dtypes
    bf16 = ml_dtypes.bfloat16

    x = np.asarray(inputs["x"], np.float32)
    Wq = np.asarray(inputs["Wq"], np.float32)
    bq = np.asarray(inputs["bq"], np.float32)
    Wk = np.asarray(inputs["Wk"], np.float32)
    bk = np.asarray(inputs["bk"], np.float32)
    Wv = np.asarray(inputs["Wv"], np.float32)
    bv = np.asarray(inputs["bv"], np.float32)
    Wo = np.asarray(inputs["Wo"], np.float32)
    bo = np.asarray(inputs["bo"], np.float32)

    scale = np.float32(1.0 / np.sqrt(HD))
    nc = _get_nc()

    def packw(wT, j):      # [128*j, cols] -> [128, j*cols] chunk-packed
        cols = wT.shape[1]
        return np.ascontiguousarray(
            wT.reshape(j, 128, cols).transpose(1, 0, 2).reshape(128, j * cols))

    in_maps = []
    bvwo = []     # host-side bv @ woT rows, one per core
    for c in range(N_CORES):
        b, g = c // 2, c % 2
        cols = slice(J * g, J * (g + 1))
        woTs = np.ascontiguousarray(Wo[:, cols].T)
        xTb = np.ascontiguousarray(x[b].T)
        in_maps.append({
            "xTp": np.ascontiguousarray(
                xTb.reshape(8, 128, S).transpose(1, 0, 2)).astype(bf16),
            "wqTp": (packw(Wq[cols, :].T, 8) * scale).astype(bf16),
            "wkTp": packw(Wk[cols, :].T, 8).astype(bf16),
            "wvTp": packw(Wv[cols, :].T, 8).astype(bf16),
            "bq": np.ascontiguousarray(bq[cols]) * scale,
            "bk": np.ascontiguousarray(bk[cols]),
            "woTp": packw(woTs, 4).astype(bf16),
        })
        bvwo.append(bv[cols] @ woTs)

    res = run_bass_kernel_spmd(
        nc, in_maps, list(range(N_CORES)),
        trace=bool(os.environ.get("BASS_TRACE")))
    LAST_RESULT["exec_time_ns"] = res.exec_time_ns
    LAST_RESULT["mean_exec_time_ns"] = getattr(res, "mean_exec_time_ns", None)
    LAST_RESULT["profile_json"] = res.profile_json
    it = res.instructions_and_trace
    LAST_RESULT["trace_path"] = it[1] if it else None
    LAST_RESULT["insts"] = it[0] if it else None

    B = x.shape[0]
    out = np.empty((B, S, DF), np.float32)
    for b in range(B):
        out[b] = (np.asarray(res.results[2 * b]["out"], np.float32)
                  + np.asarray(res.results[2 * b + 1]["out"], np.float32)
                  + bvwo[2 * b][None, :] + bvwo[2 * b + 1][None, :]
                  + bo[None, :])
    return out
